# revision 10
# baseline (speedup 1.0000x reference)
"""Self-contained Trainium2 Bass kernel for nn_EpisodicMemory_80144089743477.

kernel(**inputs) takes FULL unsharded inputs (as produced by setup_inputs())
and returns (y_em, new_K, new_V, new_S, new_age), sharding the BS=8 stream
dim across the 8 NeuronCores (one stream per core, SPMD).
"""
import os
import numpy as np

import concourse.bacc as bacc
import concourse.tile as tile
from concourse import mybir
from concourse.bass_utils import run_bass_kernel_spmd
from concourse.masks import make_identity

# Problem shapes (hardcoded per contract)
BS, B, M, D, N = 8, 4, 2048, 128, 1024
NCH = N // 128   # 8 n-chunks
MCH = M // 128   # 16 m-chunks
S_MAX = 3.0
BUDGET = 32.0
N_STEPS = 2

F32 = mybir.dt.float32
F32R = mybir.dt.float32r
BF16 = mybir.dt.bfloat16
AF = mybir.ActivationFunctionType
ALU = mybir.AluOpType

# packed fp32 input blob layout (per partition, in f32 elements)
PK_KB, PK_VB, PK_SEED, PK_WC = 0, 2048, 4096, 5120
PK_NOV = 6144
PK_SB = PK_NOV + NCH
PK_AGE = PK_SB + MCH
PK_MB = PK_AGE + MCH
PK_ACT = PK_MB + MCH
PK_GN = PK_ACT + MCH
PK_W1 = PK_GN + 1
PK_W2 = PK_W1 + 1
PK_GBH = PK_W2 + 1
PK_F_LEN = PK_GBH + 1
# packed fp32r blob: kbt [M] | vb_r [MCH*D] | seed_r [N]
PKR_KBT, PKR_VB, PKR_SEED = 0, M, M + MCH * D
PKR_LEN = M + MCH * D + N

_CACHE = {}


def _build(inv_tau: float, inv_tau_w: float):
    nc = bacc.Bacc("TRN2", target_bir_lowering=False, debug=False, num_devices=BS)

    big_f_e = nc.dram_tensor("big_f", [128, PK_F_LEN], F32, kind="ExternalInput").ap()
    big_r_e = nc.dram_tensor("big_r", [128, PKR_LEN], F32, kind="ExternalInput").ap()

    def outp(name, shape):
        return nc.dram_tensor(name, shape, F32, kind="ExternalOutput").ap()

    yem_o = outp("yemt", [D, N])
    nk_o = outp("nk", [128, MCH, D])
    nv_o = outp("nv", [128, MCH, D])
    ns_o = outp("ns", [128, MCH])
    na_o = outp("na", [128, MCH])

    with tile.TileContext(nc) as tc:
        import contextlib
        with contextlib.ExitStack() as ctx:
            consts = ctx.enter_context(tc.tile_pool(name="consts", bufs=1))
            work = ctx.enter_context(tc.tile_pool(name="work", bufs=1))
            ets = ctx.enter_context(tc.tile_pool(name="ets", bufs=3))
            blends = ctx.enter_context(tc.tile_pool(name="blends", bufs=3))

            # ---------------- loads (2 packed DMAs) ----------------
            big_f = consts.tile([128, PK_F_LEN], F32)
            nc.sync.dma_start(out=big_f, in_=big_f_e[:, :])
            big_r = consts.tile([128, PKR_LEN], F32R)
            nc.gpsimd.dma_start(out=big_r, in_=big_r_e[:, :])

            kb_f = big_f[:, PK_KB:PK_KB + MCH * D].rearrange("p (c d) -> p c d", c=MCH)
            vb_f = big_f[:, PK_VB:PK_VB + MCH * D].rearrange("p (c d) -> p c d", c=MCH)
            seed_f = big_f[:, PK_SEED:PK_SEED + N]
            wcand = big_f[:, PK_WC:PK_WC + NCH * D].rearrange("p (c d) -> p c d", c=NCH)
            nov_t = big_f[:, PK_NOV:PK_NOV + NCH]
            sb_t = big_f[:, PK_SB:PK_SB + MCH]
            age_t = big_f[:, PK_AGE:PK_AGE + MCH]
            mb_t = big_f[:, PK_MB:PK_MB + MCH]
            act_t = big_f[:, PK_ACT:PK_ACT + MCH]
            gn_t = big_f[:, PK_GN:PK_GN + 1]
            w1_t = big_f[:, PK_W1:PK_W1 + 1]
            w2_t = big_f[:, PK_W2:PK_W2 + 1]
            gbh_t = big_f[:, PK_GBH:PK_GBH + 1]

            kbt_r = big_r[:, PKR_KBT:PKR_KBT + M]
            vb_r = big_r[:, PKR_VB:PKR_VB + MCH * D].rearrange("p (c d) -> p c d", c=MCH)
            seed_r = big_r[:, PKR_SEED:PKR_SEED + N]

            ones_f = consts.tile([128, 128], F32)
            nc.vector.memset(ones_f, 1.0)
            ones_r = consts.tile([128, 128], F32R)
            nc.vector.tensor_copy(ones_r, ones_f)
            ident = consts.tile([128, 128], F32)
            make_identity(nc, ident)

            wnorm = consts.tile([128, NCH, D], F32)
            wnormt_r = consts.tile([D, N], F32R)
            er_t = consts.tile([128, NCH, M], BF16)
            wt_t = consts.tile([128, NCH, 257], BF16)

            # ---------------- w_norm (natural layout) ----------------
            ssq = work.tile([128, NCH], F32, tag="ssq")
            scrap = work.tile([128, D], F32, tag="scrap")
            for i in range(NCH):
                nc.scalar.activation(out=scrap, in_=wcand[:, i], func=AF.Square,
                                     bias=0.0, scale=1.0,
                                     accum_out=ssq[:, i:i + 1])
            nrm = work.tile([128, NCH], F32, tag="nrm")
            nc.scalar.activation(out=nrm, in_=ssq, func=AF.Sqrt, bias=0.0, scale=1.0)
            nrmg = work.tile([128, NCH], F32, tag="nrmg")
            nc.vector.tensor_scalar(out=nrmg, in0=nrm, scalar1=1e-12, scalar2=None,
                                    op0=ALU.max)
            rninv = work.tile([128, NCH], F32, tag="rninv")
            nc.vector.reciprocal(rninv, nrmg)
            for i in range(NCH):
                nc.vector.tensor_scalar(out=wnorm[:, i], in0=wcand[:, i],
                                        scalar1=rninv[:, i:i + 1], scalar2=None,
                                        op0=ALU.mult)
            with tc.tile_pool(name="tpp", bufs=2, space="PSUM") as tpp:
                for i in range(NCH):
                    tp = tpp.tile([128, 128], F32, tag="tp")
                    nc.tensor.transpose(tp, wnorm[:, i], ident)
                    nc.vector.tensor_copy(wnormt_r[:, i * 128:(i + 1) * 128], tp)

            # ---------------- trail read (2 steps, T layout) ----------------
            y_cur = seed_r
            for step in range(N_STEPS):
                with tc.tile_pool(name=f"trailp{step}", bufs=1, space="PSUM") as trailp, \
                     tc.tile_pool(name=f"scp{step}", bufs=2, space="PSUM") as scp:
                    du_ps = trailp.tile([D, N], F32, tag="du")
                    rs_ps = trailp.tile([128, N], F32, tag="rs")
                    # software-pipelined: QK(j) runs while exp(j-1) and
                    # AV(j-1) drain, so PE never waits on ACT.
                    et_q = [None] * MCH

                    def _av(jj):
                        for t in range(2):
                            nc.tensor.matmul(
                                du_ps[:, t * 512:(t + 1) * 512],
                                vb_r[:, jj],
                                et_q[jj][:, t * 512:(t + 1) * 512],
                                start=(jj == 0), stop=(jj == MCH - 1))
                            nc.tensor.matmul(
                                rs_ps[:, t * 512:(t + 1) * 512],
                                ones_r,
                                et_q[jj][:, t * 512:(t + 1) * 512],
                                start=(jj == 0), stop=(jj == MCH - 1))

                    for j in range(MCH):
                        sc = scp.tile([128, N], F32, tag="sc")
                        for t in range(2):
                            nc.tensor.matmul(
                                sc[:, t * 512:(t + 1) * 512],
                                kbt_r[:, j * 128:(j + 1) * 128],
                                y_cur[:, t * 512:(t + 1) * 512],
                                start=True, stop=True)
                        if j > 0:
                            _av(j - 1)
                        et = ets.tile([128, N], F32R, tag="et")
                        nc.scalar.activation(out=et, in_=sc, func=AF.Exp,
                                             bias=mb_t[:, j:j + 1],
                                             scale=inv_tau)
                        et_q[j] = et
                    _av(MCH - 1)
                    # normalize + gate
                    rs_sb = work.tile([128, N], F32, tag="rs_sb")
                    nc.vector.tensor_copy(rs_sb, rs_ps)
                    rcp = work.tile([128, N], F32, tag="rcp")
                    nc.vector.reciprocal(rcp, rs_sb)
                    delta = work.tile([D, N], F32, tag="delta")
                    nc.vector.tensor_mul(delta, du_ps, rcp)
                    za = work.tile([D, N], F32, tag="za")
                    nc.vector.tensor_scalar(out=za, in0=y_cur.bitcast(F32),
                                            scalar1=w1_t, scalar2=None, op0=ALU.mult)
                    zb = work.tile([D, N], F32, tag="zb")
                    nc.vector.tensor_scalar(out=zb, in0=delta, scalar1=w2_t,
                                            scalar2=None, op0=ALU.mult)
                    zz = work.tile([D, N], F32, tag="zz")
                    nc.vector.tensor_add(zz, za, zb)
                    gg = work.tile([D, N], F32, tag="gg")
                    nc.scalar.activation(out=gg, in_=zz, func=AF.Tanh,
                                         bias=gbh_t, scale=0.5)
                    uu = work.tile([D, N], F32, tag="uu")
                    nc.vector.tensor_mul(uu, gg, delta)
                    vv = work.tile([D, N], F32, tag="vv")
                    nc.vector.tensor_add(vv, uu, delta)
                    hh = work.tile([D, N], F32, tag="hh")
                    nc.vector.tensor_scalar(out=hh, in0=vv, scalar1=0.5,
                                            scalar2=None, op0=ALU.mult)
                    if step < N_STEPS - 1:
                        y_next = consts.tile([D, N], F32R)
                        nc.vector.tensor_add(y_next, y_cur.bitcast(F32), hh)
                        y_cur = y_next
                    else:
                        y2f = work.tile([D, N], F32, tag="y2f")
                        nc.vector.tensor_add(y2f, y_cur.bitcast(F32), hh)
                        y_em_f = work.tile([D, N], F32, tag="yem")
                        nc.vector.tensor_sub(y_em_f, y2f, seed_f)
                        nc.sync.dma_start(out=yem_o[:, :], in_=y_em_f)

            # ---------------- route (natural layout) ----------------
            rsum = work.tile([128, NCH], F32, tag="rsum")
            with tc.tile_pool(name="routep", bufs=2, space="PSUM") as routep:
                for i in range(NCH):
                    rt = routep.tile([128, M], F32, tag="rt")
                    for mt in range(4):
                        nc.tensor.matmul(
                            rt[:, mt * 512:(mt + 1) * 512],
                            wnormt_r[:, i * 128:(i + 1) * 128],
                            kbt_r[:, mt * 512:(mt + 1) * 512],
                            start=True, stop=True)
                    nc.scalar.activation(out=er_t[:, i], in_=rt, func=AF.Exp,
                                         bias=0.0, scale=inv_tau_w,
                                         accum_out=rsum[:, i:i + 1])
            rphi = work.tile([128, NCH], F32, tag="rphi")
            nc.vector.reciprocal(rphi, rsum)
            phi = work.tile([128, NCH], F32, tag="phi")
            nc.vector.tensor_mul(phi, rphi, nov_t)
            for i in range(NCH):
                nc.vector.tensor_scalar(out=wt_t[:, i, 0:128], in0=wnorm[:, i],
                                        scalar1=phi[:, i:i + 1], scalar2=None,
                                        op0=ALU.mult)
                nc.vector.tensor_scalar(out=wt_t[:, i, 128:256], in0=wcand[:, i],
                                        scalar1=phi[:, i:i + 1], scalar2=None,
                                        op0=ALU.mult)
                nc.vector.tensor_copy(wt_t[:, i, 256:257], phi[:, i:i + 1])

            # ---------------- update matmuls: stage uK/uV/den/ssk ----------------
            uall = consts.tile([128, MCH, 257], F32)
            ssks = work.tile([128, MCH], F32, tag="ssks")
            with tc.tile_pool(name="updp", bufs=4, space="PSUM") as updp:
                for j in range(MCH):
                    up = updp.tile([128, 257], F32, tag="up")
                    for i in range(NCH):
                        nc.tensor.matmul(up, er_t[:, i, j * 128:(j + 1) * 128],
                                         wt_t[:, i], start=(i == 0),
                                         stop=(i == NCH - 1))
                    nc.vector.tensor_copy(uall[:, j], up)
                    scrapu = blends.tile([128, D], F32, tag="scrapu")
                    nc.scalar.activation(out=scrapu, in_=up[:, 0:128],
                                         func=AF.Square, bias=0.0, scale=1.0,
                                         accum_out=ssks[:, j:j + 1])
            uks = uall[:, :, 0:128]
            uvs = uall[:, :, 128:256]
            dens = uall[:, :, 256]

            # batched [128, MCH] blend scalars
            al = work.tile([128, MCH], F32, tag="al")
            nc.vector.tensor_scalar(out=al, in0=dens, scalar1=gn_t, scalar2=1.0,
                                    op0=ALU.mult, op1=ALU.min)
            apv = work.tile([128, MCH], F32, tag="apv")
            nc.vector.tensor_mul(apv, al, act_t)
            oma = work.tile([128, MCH], F32, tag="oma")
            nc.vector.tensor_scalar(out=oma, in0=apv, scalar1=-1.0, scalar2=1.0,
                                    op0=ALU.mult, op1=ALU.add)
            dng = work.tile([128, MCH], F32, tag="dng")
            nc.vector.tensor_scalar(out=dng, in0=dens, scalar1=1e-8, scalar2=None,
                                    op0=ALU.max)
            rdn = work.tile([128, MCH], F32, tag="rdn")
            nc.vector.reciprocal(rdn, dng)
            rdna = work.tile([128, MCH], F32, tag="rdna")
            nc.vector.tensor_mul(rdna, rdn, apv)
            nrmk = work.tile([128, MCH], F32, tag="nrmk")
            nc.scalar.activation(out=nrmk, in_=ssks, func=AF.Sqrt, bias=0.0, scale=1.0)
            nrmkg = work.tile([128, MCH], F32, tag="nrmkg")
            nc.vector.tensor_scalar(out=nrmkg, in0=nrmk, scalar1=1e-12, scalar2=None,
                                    op0=ALU.max)
            rnk = work.tile([128, MCH], F32, tag="rnk")
            nc.vector.reciprocal(rnk, nrmkg)
            rna = work.tile([128, MCH], F32, tag="rna")
            nc.vector.tensor_mul(rna, rnk, apv)
            # new_S / new_age
            spv = work.tile([128, MCH], F32, tag="spv")
            nc.vector.tensor_add(spv, sb_t, apv)
            pres = work.tile([128, MCH], F32, tag="pres")
            nc.vector.tensor_scalar(out=pres, in0=spv, scalar1=0.0, scalar2=S_MAX,
                                    op0=ALU.max, op1=ALU.min)
            nage = work.tile([128, MCH], F32, tag="nage")
            nc.vector.tensor_mul(nage, age_t, oma)
            nc.sync.dma_start(out=na_o[:, :], in_=nage)
            rows = work.tile([128, 1], F32, tag="rows")
            nc.vector.reduce_sum(rows, pres, axis=mybir.AxisListType.X)
            with tc.tile_pool(name="totp", bufs=1, space="PSUM") as totp:
                tot_ps = totp.tile([128, 1], F32, tag="tot")
                nc.tensor.matmul(tot_ps, ones_f, rows, start=True, stop=True)
                totg = work.tile([128, 1], F32, tag="totg")
                nc.vector.tensor_scalar(out=totg, in0=tot_ps, scalar1=1e-8,
                                        scalar2=None, op0=ALU.max)
            rtot = work.tile([128, 1], F32, tag="rtot")
            nc.vector.reciprocal(rtot, totg)
            sc32 = work.tile([128, 1], F32, tag="sc32")
            nc.vector.tensor_scalar(out=sc32, in0=rtot, scalar1=BUDGET,
                                    scalar2=1.0, op0=ALU.mult, op1=ALU.min)
            nss = work.tile([128, MCH], F32, tag="nss")
            nc.vector.tensor_scalar(out=nss, in0=pres, scalar1=sc32,
                                    scalar2=None, op0=ALU.mult)
            nc.sync.dma_start(out=ns_o[:, :], in_=nss)

            # ---------------- blends (batched stores) ----------------
            nks = consts.tile([128, MCH, D], F32)
            nvs = consts.tile([128, MCH, D], F32)
            for j in range(MCH):
                ek = blends.tile([128, D], F32, tag="ek")
                nc.vector.tensor_scalar(out=ek, in0=uks[:, j],
                                        scalar1=rna[:, j:j + 1],
                                        scalar2=None, op0=ALU.mult)
                fk = blends.tile([128, D], F32, tag="fk")
                nc.gpsimd.tensor_scalar(out=fk, in0=kb_f[:, j],
                                        scalar1=oma[:, j:j + 1],
                                        scalar2=None, op0=ALU.mult)
                nc.vector.tensor_add(nks[:, j], ek, fk)
                ev = blends.tile([128, D], F32, tag="ev")
                nc.vector.tensor_scalar(out=ev, in0=uvs[:, j],
                                        scalar1=rdna[:, j:j + 1],
                                        scalar2=None, op0=ALU.mult)
                fv = blends.tile([128, D], F32, tag="fv")
                nc.gpsimd.tensor_scalar(out=fv, in0=vb_f[:, j],
                                        scalar1=oma[:, j:j + 1],
                                        scalar2=None, op0=ALU.mult)
                nc.gpsimd.tensor_add(nvs[:, j], ev, fv)
            nc.sync.dma_start(out=nk_o[:, :, :], in_=nks)
            nc.sync.dma_start(out=nv_o[:, :, :], in_=nvs)

    nc.compile()
    return nc


def _softplus(x):
    return float(np.log1p(np.exp(-abs(x))) + max(x, 0.0))


def kernel(seed, w_cand, novelty, g_em, em_K, em_V, em_S, em_age,
           w1, w2, gate_bias, raw_tau, raw_tau_w, b):
    bi = int(b)
    seed = np.asarray(seed, np.float32)
    w_cand = np.asarray(w_cand, np.float32)
    novelty = np.asarray(novelty, np.float32)
    g_em = np.asarray(g_em, np.float32)
    Kb = np.asarray(em_K, np.float32)[:, bi]    # [BS, M, D]
    Vb = np.asarray(em_V, np.float32)[:, bi]
    Sb = np.asarray(em_S, np.float32)[:, bi]    # [BS, M]
    ageb = np.asarray(em_age, np.float32)[:, bi]
    w1b = np.asarray(w1, np.float32)[bi]        # [D]
    w2b = np.asarray(w2, np.float32)[bi]
    gbb = np.asarray(gate_bias, np.float32)[bi]
    tau = _softplus(float(np.asarray(raw_tau)[bi])) + 0.1
    tau_w = _softplus(float(np.asarray(raw_tau_w)[bi])) + 0.1

    key = (round(1.0 / tau, 9), round(1.0 / tau_w, 9))
    if key not in _CACHE:
        _CACHE[key] = _build(1.0 / tau, 1.0 / tau_w)
    nc = _CACHE[key]

    in_maps = []
    for s in range(BS):
        mb = np.where(Sb[s] > 0, 0.0, -1e30).astype(np.float32)
        act = (Sb[s] > 0).astype(np.float32)
        big_f = np.empty((128, PK_F_LEN), np.float32)
        big_f[:, PK_KB:PK_KB + MCH * D] = \
            Kb[s].reshape(MCH, 128, D).transpose(1, 0, 2).reshape(128, MCH * D)
        big_f[:, PK_VB:PK_VB + MCH * D] = \
            Vb[s].reshape(MCH, 128, D).transpose(1, 0, 2).reshape(128, MCH * D)
        big_f[:, PK_SEED:PK_SEED + N] = seed[s].T
        big_f[:, PK_WC:PK_WC + NCH * D] = \
            w_cand[s].reshape(NCH, 128, D).transpose(1, 0, 2).reshape(128, NCH * D)
        big_f[:, PK_NOV:PK_NOV + NCH] = novelty[s].reshape(NCH, 128).T
        big_f[:, PK_SB:PK_SB + MCH] = Sb[s].reshape(MCH, 128).T
        big_f[:, PK_AGE:PK_AGE + MCH] = ageb[s].reshape(MCH, 128).T
        big_f[:, PK_MB:PK_MB + MCH] = mb.reshape(MCH, 128).T
        big_f[:, PK_ACT:PK_ACT + MCH] = act.reshape(MCH, 128).T
        big_f[:, PK_GN] = float(g_em[s]) / N
        big_f[:, PK_W1] = w1b
        big_f[:, PK_W2] = w2b
        big_f[:, PK_GBH] = 0.5 * gbb
        big_r = np.empty((128, PKR_LEN), np.float32)
        big_r[:, PKR_KBT:PKR_KBT + M] = Kb[s].T
        big_r[:, PKR_VB:PKR_VB + MCH * D] = big_f[:, PK_VB:PK_VB + MCH * D]
        big_r[:, PKR_SEED:PKR_SEED + N] = seed[s].T
        in_maps.append({"big_f": big_f, "big_r": big_r})

    _trace = os.environ.get("KERNEL_TRACE", "0") == "1"
    res = run_bass_kernel_spmd(nc, in_maps, list(range(BS)), trace=_trace)
    if _trace and getattr(res, "exec_time_ns", None) is not None:
        print(f"HW exec time: {res.exec_time_ns} ns")

    y_em = np.empty((BS, N, D), np.float32)
    new_K = np.empty((BS, M, D), np.float32)
    new_V = np.empty((BS, M, D), np.float32)
    new_S = np.empty((BS, M), np.float32)
    new_age = np.empty((BS, M), np.float32)
    for s in range(BS):
        r = res.results[s]
        y_em[s] = r["yemt"].T
        new_K[s] = r["nk"].transpose(1, 0, 2).reshape(M, D)
        new_V[s] = r["nv"].transpose(1, 0, 2).reshape(M, D)
        new_S[s] = r["ns"].T.reshape(M)
        new_age[s] = r["na"].T.reshape(M)
    return (y_em, new_K, new_V, new_S, new_age)


# revision 13
# speedup vs baseline: 1.0806x; 1.0806x over previous
"""Self-contained Trainium2 Bass kernel for nn_EpisodicMemory_80144089743477.

kernel(**inputs) takes FULL unsharded inputs (as produced by setup_inputs())
and returns (y_em, new_K, new_V, new_S, new_age), sharding the BS=8 stream
dim across the 8 NeuronCores (one stream per core, SPMD).
"""
import os
import numpy as np

import concourse.bacc as bacc
import concourse.tile as tile
from concourse import mybir
from concourse.bass_utils import run_bass_kernel_spmd
from concourse.masks import make_identity

# Problem shapes (hardcoded per contract)
BS, B, M, D, N = 8, 4, 2048, 128, 1024
NCH = N // 128   # 8 n-chunks
MCH = M // 128   # 16 m-chunks
S_MAX = 3.0
BUDGET = 32.0
N_STEPS = 2

F32 = mybir.dt.float32
F32R = mybir.dt.float32r
BF16 = mybir.dt.bfloat16
AF = mybir.ActivationFunctionType
ALU = mybir.AluOpType

# packed fp32 input blob layout (per partition, in f32 elements)
PK_KB, PK_VB, PK_SEED, PK_WC = 0, 2048, 4096, 5120
PK_NOV = 6144
PK_SB = PK_NOV + NCH
PK_AGE = PK_SB + MCH
PK_MB = PK_AGE + MCH
PK_ACT = PK_MB + MCH
PK_GN = PK_ACT + MCH
PK_W1 = PK_GN + 1
PK_W2 = PK_W1 + 1
PK_GBH = PK_W2 + 1
PK_F_LEN = PK_GBH + 1
# packed fp32r blob: kbt [M] | vb_r [MCH*D] | seed_r [N]
PKR_KBT, PKR_VB, PKR_SEED = 0, M, M + MCH * D
PKR_LEN = M + MCH * D + N

_CACHE = {}


def _build(inv_tau: float, inv_tau_w: float):
    nc = bacc.Bacc("TRN2", target_bir_lowering=False, debug=False, num_devices=BS)

    big_f_e = nc.dram_tensor("big_f", [128, PK_F_LEN], F32, kind="ExternalInput").ap()
    big_r_e = nc.dram_tensor("big_r", [128, PKR_LEN], F32, kind="ExternalInput").ap()

    def outp(name, shape):
        return nc.dram_tensor(name, shape, F32, kind="ExternalOutput").ap()

    yem_o = outp("yemt", [D, N])
    nk_o = outp("nk", [128, MCH, D])
    nv_o = outp("nv", [128, MCH, D])
    ns_o = outp("ns", [128, MCH])
    na_o = outp("na", [128, MCH])

    with tile.TileContext(nc) as tc:
        import contextlib
        with contextlib.ExitStack() as ctx:
            consts = ctx.enter_context(tc.tile_pool(name="consts", bufs=1))
            work = ctx.enter_context(tc.tile_pool(name="work", bufs=1))
            ets = ctx.enter_context(tc.tile_pool(name="ets", bufs=3))
            blends = ctx.enter_context(tc.tile_pool(name="blends", bufs=3))

            # ---------------- loads (need-ordered split DMAs) ----------------
            # critical prefix first: smalls (mask bias etc), kbt^T + seed^T
            # (fp32r, gpsimd cast queue), so the first QK can start ~5us in.
            sm_len = PK_F_LEN - PK_NOV
            sm_t = consts.tile([128, sm_len], F32)
            nc.sync.dma_start(out=sm_t, in_=big_f_e[:, PK_NOV:PK_F_LEN])
            kbt_f = consts.tile([D, M], F32)
            nc.sync.dma_start(out=kbt_f, in_=big_r_e[:, PKR_KBT:PKR_KBT + M])
            kbt_r = consts.tile([D, M], F32R)
            nc.vector.tensor_copy(kbt_r, kbt_f)
            seed_r = consts.tile([D, N], F32R)
            nc.gpsimd.dma_start(out=seed_r, in_=big_r_e[:, PKR_SEED:PKR_SEED + N])
            vb_rt = consts.tile([128, MCH * D], F32R)
            nc.gpsimd.dma_start(out=vb_rt, in_=big_r_e[:, PKR_VB:PKR_VB + MCH * D])
            sdwc_t = consts.tile([128, 2048], F32)
            nc.sync.dma_start(out=sdwc_t, in_=big_f_e[:, PK_SEED:PK_SEED + 2048])
            kbf_t = consts.tile([128, MCH * D], F32)
            nc.sync.dma_start(out=kbf_t, in_=big_f_e[:, PK_KB:PK_KB + MCH * D])
            vbf_t = consts.tile([128, MCH * D], F32)
            nc.sync.dma_start(out=vbf_t, in_=big_f_e[:, PK_VB:PK_VB + MCH * D])

            kb_f = kbf_t.rearrange("p (c d) -> p c d", c=MCH)
            vb_f = vbf_t.rearrange("p (c d) -> p c d", c=MCH)
            seed_f = sdwc_t[:, 0:N]
            wcand = sdwc_t[:, N:N + NCH * D].rearrange("p (c d) -> p c d", c=NCH)
            nov_t = sm_t[:, PK_NOV - PK_NOV:PK_NOV - PK_NOV + NCH]
            sb_t = sm_t[:, PK_SB - PK_NOV:PK_SB - PK_NOV + MCH]
            age_t = sm_t[:, PK_AGE - PK_NOV:PK_AGE - PK_NOV + MCH]
            mb_t = sm_t[:, PK_MB - PK_NOV:PK_MB - PK_NOV + MCH]
            act_t = sm_t[:, PK_ACT - PK_NOV:PK_ACT - PK_NOV + MCH]
            gn_t = sm_t[:, PK_GN - PK_NOV:PK_GN - PK_NOV + 1]
            w1_t = sm_t[:, PK_W1 - PK_NOV:PK_W1 - PK_NOV + 1]
            w2_t = sm_t[:, PK_W2 - PK_NOV:PK_W2 - PK_NOV + 1]
            gbh_t = sm_t[:, PK_GBH - PK_NOV:PK_GBH - PK_NOV + 1]

            vb_r = vb_rt.rearrange("p (c d) -> p c d", c=MCH)

            ones_f = consts.tile([128, 128], F32)
            nc.vector.memset(ones_f, 1.0)
            ones_r = consts.tile([128, 128], F32R)
            nc.vector.tensor_copy(ones_r, ones_f)
            ident = consts.tile([128, 128], F32)
            make_identity(nc, ident)

            wnorm = consts.tile([128, NCH, D], F32)
            wnormt_r = consts.tile([D, N], F32R)
            er_t = consts.tile([128, NCH, M], BF16)
            wt_t = consts.tile([128, NCH, 257], BF16)

            # ---------------- w_norm (natural layout) ----------------
            ssq = work.tile([128, NCH], F32, tag="ssq")
            scrap = work.tile([128, D], F32, tag="scrap")
            for i in range(NCH):
                nc.scalar.activation(out=scrap, in_=wcand[:, i], func=AF.Square,
                                     bias=0.0, scale=1.0,
                                     accum_out=ssq[:, i:i + 1])
            nrm = work.tile([128, NCH], F32, tag="nrm")
            nc.scalar.activation(out=nrm, in_=ssq, func=AF.Sqrt, bias=0.0, scale=1.0)
            nrmg = work.tile([128, NCH], F32, tag="nrmg")
            nc.vector.tensor_scalar(out=nrmg, in0=nrm, scalar1=1e-12, scalar2=None,
                                    op0=ALU.max)
            rninv = work.tile([128, NCH], F32, tag="rninv")
            nc.vector.reciprocal(rninv, nrmg)
            for i in range(NCH):
                nc.vector.tensor_scalar(out=wnorm[:, i], in0=wcand[:, i],
                                        scalar1=rninv[:, i:i + 1], scalar2=None,
                                        op0=ALU.mult)
            with tc.tile_pool(name="tpp", bufs=2, space="PSUM") as tpp:
                for i in range(NCH):
                    tp = tpp.tile([128, 128], F32, tag="tp")
                    nc.tensor.transpose(tp, wnorm[:, i], ident)
                    nc.vector.tensor_copy(wnormt_r[:, i * 128:(i + 1) * 128], tp)

            # ---------------- trail read (2 steps, T layout) ----------------
            y_cur = seed_r
            for step in range(N_STEPS):
                with tc.tile_pool(name=f"trailp{step}", bufs=1, space="PSUM") as trailp, \
                     tc.tile_pool(name=f"scp{step}", bufs=2, space="PSUM") as scp:
                    du_ps = trailp.tile([D, N], F32, tag="du")
                    rs_ps = trailp.tile([128, N], F32, tag="rs")
                    # software-pipelined: QK(j) runs while exp(j-1) and
                    # AV(j-1) drain, so PE never waits on ACT.
                    et_q = [None] * MCH

                    def _av(jj):
                        for t in range(2):
                            nc.tensor.matmul(
                                du_ps[:, t * 512:(t + 1) * 512],
                                vb_r[:, jj],
                                et_q[jj][:, t * 512:(t + 1) * 512],
                                start=(jj == 0), stop=(jj == MCH - 1))
                            nc.tensor.matmul(
                                rs_ps[:, t * 512:(t + 1) * 512],
                                ones_r,
                                et_q[jj][:, t * 512:(t + 1) * 512],
                                start=(jj == 0), stop=(jj == MCH - 1))

                    for j in range(MCH):
                        sc = scp.tile([128, N], F32, tag="sc")
                        for t in range(2):
                            nc.tensor.matmul(
                                sc[:, t * 512:(t + 1) * 512],
                                kbt_r[:, j * 128:(j + 1) * 128],
                                y_cur[:, t * 512:(t + 1) * 512],
                                start=True, stop=True)
                        if j > 0:
                            _av(j - 1)
                        et = ets.tile([128, N], F32R, tag="et")
                        nc.scalar.activation(out=et, in_=sc, func=AF.Exp,
                                             bias=mb_t[:, j:j + 1],
                                             scale=inv_tau)
                        et_q[j] = et
                    za = work.tile([D, N], F32, tag="za")
                    nc.vector.tensor_scalar(out=za, in0=y_cur.bitcast(F32),
                                            scalar1=w1_t, scalar2=None, op0=ALU.mult)
                    _av(MCH - 1)
                    # normalize + gate
                    rs_sb = work.tile([128, N], F32, tag="rs_sb")
                    nc.vector.tensor_copy(rs_sb, rs_ps)
                    rcp = work.tile([128, N], F32, tag="rcp")
                    nc.vector.reciprocal(rcp, rs_sb)
                    delta = work.tile([D, N], F32, tag="delta")
                    nc.vector.tensor_mul(delta, du_ps, rcp)
                    zb = work.tile([D, N], F32, tag="zb")
                    nc.vector.tensor_scalar(out=zb, in0=delta, scalar1=w2_t,
                                            scalar2=None, op0=ALU.mult)
                    zz = work.tile([D, N], F32, tag="zz")
                    nc.vector.tensor_add(zz, za, zb)
                    gg = work.tile([D, N], F32, tag="gg")
                    nc.scalar.activation(out=gg, in_=zz, func=AF.Tanh,
                                         bias=gbh_t, scale=0.5)
                    uu = work.tile([D, N], F32, tag="uu")
                    nc.vector.tensor_mul(uu, gg, delta)
                    vv = work.tile([D, N], F32, tag="vv")
                    nc.vector.tensor_add(vv, uu, delta)
                    hh = work.tile([D, N], F32, tag="hh")
                    nc.vector.tensor_scalar(out=hh, in0=vv, scalar1=0.5,
                                            scalar2=None, op0=ALU.mult)
                    if step < N_STEPS - 1:
                        y_next = consts.tile([D, N], F32R)
                        nc.vector.tensor_add(y_next, y_cur.bitcast(F32), hh)
                        y_cur = y_next
                    else:
                        y2f = work.tile([D, N], F32, tag="y2f")
                        nc.vector.tensor_add(y2f, y_cur.bitcast(F32), hh)
                        y_em_f = work.tile([D, N], F32, tag="yem")
                        nc.vector.tensor_sub(y_em_f, y2f, seed_f)
                        nc.sync.dma_start(out=yem_o[:, :], in_=y_em_f)

            # ---------------- route (natural layout) ----------------
            rsum = work.tile([128, NCH], F32, tag="rsum")
            with tc.tile_pool(name="routep", bufs=2, space="PSUM") as routep:
                for i in range(NCH):
                    rt = routep.tile([128, M], F32, tag="rt")
                    for mt in range(4):
                        nc.tensor.matmul(
                            rt[:, mt * 512:(mt + 1) * 512],
                            wnormt_r[:, i * 128:(i + 1) * 128],
                            kbt_r[:, mt * 512:(mt + 1) * 512],
                            start=True, stop=True)
                    nc.scalar.activation(out=er_t[:, i], in_=rt, func=AF.Exp,
                                         bias=0.0, scale=inv_tau_w,
                                         accum_out=rsum[:, i:i + 1])
            rphi = work.tile([128, NCH], F32, tag="rphi")
            nc.vector.reciprocal(rphi, rsum)
            phi = work.tile([128, NCH], F32, tag="phi")
            nc.vector.tensor_mul(phi, rphi, nov_t)
            for i in range(NCH):
                nc.vector.tensor_scalar(out=wt_t[:, i, 0:128], in0=wnorm[:, i],
                                        scalar1=phi[:, i:i + 1], scalar2=None,
                                        op0=ALU.mult)
                nc.vector.tensor_scalar(out=wt_t[:, i, 128:256], in0=wcand[:, i],
                                        scalar1=phi[:, i:i + 1], scalar2=None,
                                        op0=ALU.mult)
                nc.vector.tensor_copy(wt_t[:, i, 256:257], phi[:, i:i + 1])

            # ---------------- update matmuls: stage uK/uV/den/ssk ----------------
            uall = consts.tile([128, MCH, 257], F32)
            ssks = work.tile([128, MCH], F32, tag="ssks")
            al = work.tile([128, MCH], F32, tag="al")
            apv = work.tile([128, MCH], F32, tag="apv")
            oma = work.tile([128, MCH], F32, tag="oma")
            dng = work.tile([128, MCH], F32, tag="dng")
            rdn = work.tile([128, MCH], F32, tag="rdn")
            rdna = work.tile([128, MCH], F32, tag="rdna")
            nrmk = work.tile([128, MCH], F32, tag="nrmk")
            nrmkg = work.tile([128, MCH], F32, tag="nrmkg")
            rnk = work.tile([128, MCH], F32, tag="rnk")
            rna = work.tile([128, MCH], F32, tag="rna")
            spv = work.tile([128, MCH], F32, tag="spv")
            pres = work.tile([128, MCH], F32, tag="pres")
            nage = work.tile([128, MCH], F32, tag="nage")
            nks = consts.tile([128, MCH, D], F32)
            nvs = consts.tile([128, MCH, D], F32)
            uks = uall[:, :, 0:128]
            uvs = uall[:, :, 128:256]
            dens = uall[:, :, 256]
            HB = MCH // 2
            with tc.tile_pool(name="updp", bufs=4, space="PSUM") as updp:
                for half in range(2):
                    j0, j1 = half * HB, (half + 1) * HB
                    for j in range(j0, j1):
                        up = updp.tile([128, 257], F32, tag="up")
                        for i in range(NCH):
                            nc.tensor.matmul(up, er_t[:, i, j * 128:(j + 1) * 128],
                                             wt_t[:, i], start=(i == 0),
                                             stop=(i == NCH - 1))
                        nc.vector.tensor_copy(uall[:, j], up)
                        scrapu = blends.tile([128, D], F32, tag="scrapu")
                        nc.scalar.activation(out=scrapu, in_=up[:, 0:128],
                                             func=AF.Square, bias=0.0, scale=1.0,
                                             accum_out=ssks[:, j:j + 1])
                    sl = slice(j0, j1)
                    nc.vector.tensor_scalar(out=al[:, sl], in0=dens[:, sl],
                                            scalar1=gn_t, scalar2=1.0,
                                            op0=ALU.mult, op1=ALU.min)
                    nc.vector.tensor_mul(apv[:, sl], al[:, sl], act_t[:, sl])
                    nc.vector.tensor_scalar(out=oma[:, sl], in0=apv[:, sl],
                                            scalar1=-1.0, scalar2=1.0,
                                            op0=ALU.mult, op1=ALU.add)
                    nc.vector.tensor_scalar(out=dng[:, sl], in0=dens[:, sl],
                                            scalar1=1e-8, scalar2=None, op0=ALU.max)
                    nc.vector.reciprocal(rdn[:, sl], dng[:, sl])
                    nc.vector.tensor_mul(rdna[:, sl], rdn[:, sl], apv[:, sl])
                    nc.scalar.activation(out=nrmk[:, sl], in_=ssks[:, sl],
                                         func=AF.Sqrt, bias=0.0, scale=1.0)
                    nc.vector.tensor_scalar(out=nrmkg[:, sl], in0=nrmk[:, sl],
                                            scalar1=1e-12, scalar2=None, op0=ALU.max)
                    nc.vector.reciprocal(rnk[:, sl], nrmkg[:, sl])
                    nc.vector.tensor_mul(rna[:, sl], rnk[:, sl], apv[:, sl])
                    nc.vector.tensor_add(spv[:, sl], sb_t[:, sl], apv[:, sl])
                    nc.vector.tensor_scalar(out=pres[:, sl], in0=spv[:, sl],
                                            scalar1=0.0, scalar2=S_MAX,
                                            op0=ALU.max, op1=ALU.min)
                    nc.vector.tensor_mul(nage[:, sl], age_t[:, sl], oma[:, sl])
                    for j in range(j0, j1):
                        ek = blends.tile([128, D], F32, tag="ek")
                        nc.vector.tensor_scalar(out=ek, in0=uks[:, j],
                                                scalar1=rna[:, j:j + 1],
                                                scalar2=None, op0=ALU.mult)
                        fk = blends.tile([128, D], F32, tag="fk")
                        nc.gpsimd.tensor_scalar(out=fk, in0=kb_f[:, j],
                                                scalar1=oma[:, j:j + 1],
                                                scalar2=None, op0=ALU.mult)
                        nc.vector.tensor_add(nks[:, j], ek, fk)
                        ev = blends.tile([128, D], F32, tag="ev")
                        nc.vector.tensor_scalar(out=ev, in0=uvs[:, j],
                                                scalar1=rdna[:, j:j + 1],
                                                scalar2=None, op0=ALU.mult)
                        fv = blends.tile([128, D], F32, tag="fv")
                        nc.gpsimd.tensor_scalar(out=fv, in0=vb_f[:, j],
                                                scalar1=oma[:, j:j + 1],
                                                scalar2=None, op0=ALU.mult)
                        nc.gpsimd.tensor_add(nvs[:, j], ev, fv)
                nc.sync.dma_start(out=na_o[:, :], in_=nage)
                rows = work.tile([128, 1], F32, tag="rows")
                nc.vector.reduce_sum(rows, pres, axis=mybir.AxisListType.X)
                tot_ps = updp.tile([128, 1], F32, tag="tot")
                nc.tensor.matmul(tot_ps, ones_f, rows, start=True, stop=True)
                totg = work.tile([128, 1], F32, tag="totg")
                nc.vector.tensor_scalar(out=totg, in0=tot_ps, scalar1=1e-8,
                                        scalar2=None, op0=ALU.max)
            rtot = work.tile([128, 1], F32, tag="rtot")
            nc.vector.reciprocal(rtot, totg)
            sc32 = work.tile([128, 1], F32, tag="sc32")
            nc.vector.tensor_scalar(out=sc32, in0=rtot, scalar1=BUDGET,
                                    scalar2=1.0, op0=ALU.mult, op1=ALU.min)
            nss = work.tile([128, MCH], F32, tag="nss")
            nc.vector.tensor_scalar(out=nss, in0=pres, scalar1=sc32,
                                    scalar2=None, op0=ALU.mult)
            nc.sync.dma_start(out=ns_o[:, :], in_=nss)
            nc.sync.dma_start(out=nk_o[:, :, :], in_=nks)
            nc.sync.dma_start(out=nv_o[:, :, :], in_=nvs)

    nc.compile()
    return nc


def _softplus(x):
    return float(np.log1p(np.exp(-abs(x))) + max(x, 0.0))


def kernel(seed, w_cand, novelty, g_em, em_K, em_V, em_S, em_age,
           w1, w2, gate_bias, raw_tau, raw_tau_w, b):
    bi = int(b)
    seed = np.asarray(seed, np.float32)
    w_cand = np.asarray(w_cand, np.float32)
    novelty = np.asarray(novelty, np.float32)
    g_em = np.asarray(g_em, np.float32)
    Kb = np.asarray(em_K, np.float32)[:, bi]    # [BS, M, D]
    Vb = np.asarray(em_V, np.float32)[:, bi]
    Sb = np.asarray(em_S, np.float32)[:, bi]    # [BS, M]
    ageb = np.asarray(em_age, np.float32)[:, bi]
    w1b = np.asarray(w1, np.float32)[bi]        # [D]
    w2b = np.asarray(w2, np.float32)[bi]
    gbb = np.asarray(gate_bias, np.float32)[bi]
    tau = _softplus(float(np.asarray(raw_tau)[bi])) + 0.1
    tau_w = _softplus(float(np.asarray(raw_tau_w)[bi])) + 0.1

    key = (round(1.0 / tau, 9), round(1.0 / tau_w, 9))
    if key not in _CACHE:
        _CACHE[key] = _build(1.0 / tau, 1.0 / tau_w)
    nc = _CACHE[key]

    in_maps = []
    for s in range(BS):
        mb = np.where(Sb[s] > 0, 0.0, -1e30).astype(np.float32)
        act = (Sb[s] > 0).astype(np.float32)
        big_f = np.empty((128, PK_F_LEN), np.float32)
        big_f[:, PK_KB:PK_KB + MCH * D] = \
            Kb[s].reshape(MCH, 128, D).transpose(1, 0, 2).reshape(128, MCH * D)
        big_f[:, PK_VB:PK_VB + MCH * D] = \
            Vb[s].reshape(MCH, 128, D).transpose(1, 0, 2).reshape(128, MCH * D)
        big_f[:, PK_SEED:PK_SEED + N] = seed[s].T
        big_f[:, PK_WC:PK_WC + NCH * D] = \
            w_cand[s].reshape(NCH, 128, D).transpose(1, 0, 2).reshape(128, NCH * D)
        big_f[:, PK_NOV:PK_NOV + NCH] = novelty[s].reshape(NCH, 128).T
        big_f[:, PK_SB:PK_SB + MCH] = Sb[s].reshape(MCH, 128).T
        big_f[:, PK_AGE:PK_AGE + MCH] = ageb[s].reshape(MCH, 128).T
        big_f[:, PK_MB:PK_MB + MCH] = mb.reshape(MCH, 128).T
        big_f[:, PK_ACT:PK_ACT + MCH] = act.reshape(MCH, 128).T
        big_f[:, PK_GN] = float(g_em[s]) / N
        big_f[:, PK_W1] = w1b
        big_f[:, PK_W2] = w2b
        big_f[:, PK_GBH] = 0.5 * gbb
        big_r = np.empty((128, PKR_LEN), np.float32)
        big_r[:, PKR_KBT:PKR_KBT + M] = Kb[s].T
        big_r[:, PKR_VB:PKR_VB + MCH * D] = big_f[:, PK_VB:PK_VB + MCH * D]
        big_r[:, PKR_SEED:PKR_SEED + N] = seed[s].T
        in_maps.append({"big_f": big_f, "big_r": big_r})

    _trace = os.environ.get("KERNEL_TRACE", "0") == "1"
    res = run_bass_kernel_spmd(nc, in_maps, list(range(BS)), trace=_trace)
    if _trace and getattr(res, "exec_time_ns", None) is not None:
        print(f"HW exec time: {res.exec_time_ns} ns")

    y_em = np.empty((BS, N, D), np.float32)
    new_K = np.empty((BS, M, D), np.float32)
    new_V = np.empty((BS, M, D), np.float32)
    new_S = np.empty((BS, M), np.float32)
    new_age = np.empty((BS, M), np.float32)
    for s in range(BS):
        r = res.results[s]
        y_em[s] = r["yemt"].T
        new_K[s] = r["nk"].transpose(1, 0, 2).reshape(M, D)
        new_V[s] = r["nv"].transpose(1, 0, 2).reshape(M, D)
        new_S[s] = r["ns"].T.reshape(M)
        new_age[s] = r["na"].T.reshape(M)
    return (y_em, new_K, new_V, new_S, new_age)


# revision 18
# speedup vs baseline: 1.0949x; 1.0133x over previous
"""Self-contained Trainium2 Bass kernel for nn_EpisodicMemory_80144089743477.

kernel(**inputs) takes FULL unsharded inputs (as produced by setup_inputs())
and returns (y_em, new_K, new_V, new_S, new_age), sharding the BS=8 stream
dim across the 8 NeuronCores (one stream per core, SPMD).
"""
import os
import numpy as np

import concourse.bacc as bacc
import concourse.tile as tile
from concourse import mybir
from concourse.bass_utils import run_bass_kernel_spmd
from concourse.masks import make_identity

# Problem shapes (hardcoded per contract)
BS, B, M, D, N = 8, 4, 2048, 128, 1024
NCH = N // 128   # 8 n-chunks
MCH = M // 128   # 16 m-chunks
S_MAX = 3.0
BUDGET = 32.0
N_STEPS = 2

F32 = mybir.dt.float32
F32R = mybir.dt.float32r
BF16 = mybir.dt.bfloat16
AF = mybir.ActivationFunctionType
ALU = mybir.AluOpType

# packed fp32 input blob layout (per partition, in f32 elements)
PK_KB, PK_VB, PK_SEED, PK_WC = 0, 2048, 4096, 5120
PK_NOV = 6144
PK_SB = PK_NOV + NCH
PK_AGE = PK_SB + MCH
PK_MB = PK_AGE + MCH
PK_ACT = PK_MB + MCH
PK_GN = PK_ACT + MCH
PK_W1 = PK_GN + 1
PK_W2 = PK_W1 + 1
PK_GBH = PK_W2 + 1
PK_F_LEN = PK_GBH + 1
# packed fp32r blob: kbt [M] | vb_r [MCH*D] | seed_r [N]
PKR_KBT, PKR_VB, PKR_SEED = 0, M, M + MCH * D
PKR_LEN = M + MCH * D + N

_CACHE = {}


def _build(inv_tau: float, inv_tau_w: float):
    nc = bacc.Bacc("TRN2", target_bir_lowering=False, debug=False, num_devices=BS)

    big_f_e = nc.dram_tensor("big_f", [128, PK_F_LEN], F32, kind="ExternalInput").ap()
    big_r_e = nc.dram_tensor("big_r", [128, PKR_LEN], F32, kind="ExternalInput").ap()

    def outp(name, shape):
        return nc.dram_tensor(name, shape, F32, kind="ExternalOutput").ap()

    yem_o = outp("yemt", [D, N])
    nk_o = outp("nk", [128, MCH, D])
    nv_o = outp("nv", [128, MCH, D])
    ns_o = outp("ns", [128, MCH])
    na_o = outp("na", [128, MCH])

    with tile.TileContext(nc) as tc:
        import contextlib
        with contextlib.ExitStack() as ctx:
            consts = ctx.enter_context(tc.tile_pool(name="consts", bufs=1))
            work = ctx.enter_context(tc.tile_pool(name="work", bufs=1))
            ets = ctx.enter_context(tc.tile_pool(name="ets", bufs=3))
            blends = ctx.enter_context(tc.tile_pool(name="blends", bufs=3))

            # ---------------- loads (need-ordered split DMAs) ----------------
            # critical prefix first: smalls (mask bias etc), kbt^T + seed^T
            # (fp32r, gpsimd cast queue), so the first QK can start ~5us in.
            sm_len = PK_F_LEN - PK_NOV
            sm_t = consts.tile([128, sm_len], F32)
            nc.sync.dma_start(out=sm_t, in_=big_f_e[:, PK_NOV:PK_F_LEN])
            kbt_f = consts.tile([D, M], F32)
            nc.sync.dma_start(out=kbt_f, in_=big_r_e[:, PKR_KBT:PKR_KBT + M])
            kbt_r = consts.tile([D, M], F32R)
            nc.vector.tensor_copy(kbt_r, kbt_f)
            seed_r = consts.tile([D, N], F32R)
            nc.gpsimd.dma_start(out=seed_r, in_=big_r_e[:, PKR_SEED:PKR_SEED + N])
            vb_rt = consts.tile([128, MCH * D], F32R)
            nc.gpsimd.dma_start(out=vb_rt, in_=big_r_e[:, PKR_VB:PKR_VB + MCH * D])
            sdwc_t = consts.tile([128, 2048], F32)
            nc.sync.dma_start(out=sdwc_t, in_=big_f_e[:, PK_SEED:PK_SEED + 2048])
            kbf_t = consts.tile([128, MCH * D], F32)
            nc.sync.dma_start(out=kbf_t, in_=big_f_e[:, PK_KB:PK_KB + MCH * D])
            vbf_t = consts.tile([128, MCH * D], F32)
            nc.sync.dma_start(out=vbf_t, in_=big_f_e[:, PK_VB:PK_VB + MCH * D])

            kb_f = kbf_t.rearrange("p (c d) -> p c d", c=MCH)
            vb_f = vbf_t.rearrange("p (c d) -> p c d", c=MCH)
            seed_f = sdwc_t[:, 0:N]
            wcand = sdwc_t[:, N:N + NCH * D].rearrange("p (c d) -> p c d", c=NCH)
            nov_t = sm_t[:, PK_NOV - PK_NOV:PK_NOV - PK_NOV + NCH]
            sb_t = sm_t[:, PK_SB - PK_NOV:PK_SB - PK_NOV + MCH]
            age_t = sm_t[:, PK_AGE - PK_NOV:PK_AGE - PK_NOV + MCH]
            mb_t = sm_t[:, PK_MB - PK_NOV:PK_MB - PK_NOV + MCH]
            act_t = sm_t[:, PK_ACT - PK_NOV:PK_ACT - PK_NOV + MCH]
            gn_t = sm_t[:, PK_GN - PK_NOV:PK_GN - PK_NOV + 1]
            w1_t = sm_t[:, PK_W1 - PK_NOV:PK_W1 - PK_NOV + 1]
            w2_t = sm_t[:, PK_W2 - PK_NOV:PK_W2 - PK_NOV + 1]
            gbh_t = sm_t[:, PK_GBH - PK_NOV:PK_GBH - PK_NOV + 1]

            vb_r = vb_rt.rearrange("p (c d) -> p c d", c=MCH)

            ones_f = consts.tile([128, 128], F32)
            nc.vector.memset(ones_f, 1.0)
            ones_r = consts.tile([128, 128], F32R)
            nc.vector.tensor_copy(ones_r, ones_f)
            ident = consts.tile([128, 128], F32)
            make_identity(nc, ident)

            wnorm = consts.tile([128, NCH, D], F32)
            wnormt_r = consts.tile([D, N], F32R)
            er_t = consts.tile([128, NCH, M], BF16)
            wt_t = consts.tile([128, NCH, 257], BF16)

            # ---------------- w_norm (natural layout) ----------------
            ssq = work.tile([128, NCH], F32, tag="ssq")
            scrap = work.tile([128, D], F32, tag="scrap")
            for i in range(NCH):
                nc.scalar.activation(out=scrap, in_=wcand[:, i], func=AF.Square,
                                     bias=0.0, scale=1.0,
                                     accum_out=ssq[:, i:i + 1])
            nrm = work.tile([128, NCH], F32, tag="nrm")
            nc.scalar.activation(out=nrm, in_=ssq, func=AF.Sqrt, bias=0.0, scale=1.0)
            nrmg = work.tile([128, NCH], F32, tag="nrmg")
            nc.vector.tensor_scalar(out=nrmg, in0=nrm, scalar1=1e-12, scalar2=None,
                                    op0=ALU.max)
            rninv = work.tile([128, NCH], F32, tag="rninv")
            nc.vector.reciprocal(rninv, nrmg)
            for i in range(NCH):
                nc.vector.tensor_scalar(out=wnorm[:, i], in0=wcand[:, i],
                                        scalar1=rninv[:, i:i + 1], scalar2=None,
                                        op0=ALU.mult)
            with tc.tile_pool(name="tpp", bufs=2, space="PSUM") as tpp:
                for i in range(NCH):
                    tp = tpp.tile([128, 128], F32, tag="tp")
                    nc.tensor.transpose(tp, wnorm[:, i], ident)
                    nc.vector.tensor_copy(wnormt_r[:, i * 128:(i + 1) * 128], tp)

            # ---------------- trail read (2 steps, T layout) ----------------
            y_cur = seed_r
            for step in range(N_STEPS):
                with tc.tile_pool(name=f"trailp{step}", bufs=1, space="PSUM") as trailp, \
                     tc.tile_pool(name=f"scp{step}", bufs=2, space="PSUM") as scp:
                    du_ps = trailp.tile([D, N], F32, tag="du")
                    rs_ps = trailp.tile([128, N], F32, tag="rs")
                    # software-pipelined: QK(j) runs while exp(j-1) and
                    # AV(j-1) drain, so PE never waits on ACT.
                    et_q = [None] * MCH

                    def _av(jj):
                        for t in range(2):
                            nc.tensor.matmul(
                                du_ps[:, t * 512:(t + 1) * 512],
                                vb_r[:, jj],
                                et_q[jj][:, t * 512:(t + 1) * 512],
                                start=(jj == 0), stop=(jj == MCH - 1))
                            nc.tensor.matmul(
                                rs_ps[:, t * 512:(t + 1) * 512],
                                ones_r,
                                et_q[jj][:, t * 512:(t + 1) * 512],
                                start=(jj == 0), stop=(jj == MCH - 1))

                    for j in range(MCH):
                        sc = scp.tile([128, N], F32, tag="sc")
                        for t in range(2):
                            nc.tensor.matmul(
                                sc[:, t * 512:(t + 1) * 512],
                                kbt_r[:, j * 128:(j + 1) * 128],
                                y_cur[:, t * 512:(t + 1) * 512],
                                start=True, stop=True)
                        if j > 0:
                            _av(j - 1)
                        et = ets.tile([128, N], F32R, tag="et")
                        nc.scalar.activation(out=et, in_=sc, func=AF.Exp,
                                             bias=mb_t[:, j:j + 1],
                                             scale=inv_tau)
                        et_q[j] = et
                    za = work.tile([D, N], F32, tag="za")
                    nc.vector.tensor_scalar(out=za, in0=y_cur.bitcast(F32),
                                            scalar1=w1_t, scalar2=None, op0=ALU.mult)
                    _av(MCH - 1)
                    # normalize + gate
                    rs_sb = work.tile([128, N], F32, tag="rs_sb")
                    nc.vector.tensor_copy(rs_sb, rs_ps)
                    rcp = work.tile([128, N], F32, tag="rcp")
                    nc.vector.reciprocal(rcp, rs_sb)
                    delta = work.tile([D, N], F32, tag="delta")
                    nc.vector.tensor_mul(delta, du_ps, rcp)
                    zb = work.tile([D, N], F32, tag="zb")
                    nc.vector.tensor_scalar(out=zb, in0=delta, scalar1=w2_t,
                                            scalar2=None, op0=ALU.mult)
                    zz = work.tile([D, N], F32, tag="zz")
                    nc.vector.tensor_add(zz, za, zb)
                    gg = work.tile([D, N], F32, tag="gg")
                    nc.scalar.activation(out=gg, in_=zz, func=AF.Tanh,
                                         bias=gbh_t, scale=0.5)
                    uu = work.tile([D, N], F32, tag="uu")
                    nc.vector.tensor_mul(uu, gg, delta)
                    vv = work.tile([D, N], F32, tag="vv")
                    nc.vector.tensor_add(vv, uu, delta)
                    hh = work.tile([D, N], F32, tag="hh")
                    nc.vector.tensor_scalar(out=hh, in0=vv, scalar1=0.5,
                                            scalar2=None, op0=ALU.mult)
                    if step < N_STEPS - 1:
                        y_next = consts.tile([D, N], F32R)
                        nc.vector.tensor_add(y_next, y_cur.bitcast(F32), hh)
                        y_cur = y_next
                    else:
                        y2f = work.tile([D, N], F32, tag="y2f")
                        nc.vector.tensor_add(y2f, y_cur.bitcast(F32), hh)
                        y_em_f = work.tile([D, N], F32, tag="yem")
                        nc.vector.tensor_sub(y_em_f, y2f, seed_f)
                        nc.sync.dma_start(out=yem_o[:, :], in_=y_em_f)

            # ---------------- route (natural layout) ----------------
            rsum2 = work.tile([128, NCH, 2], F32, tag="rsum2")
            with tc.tile_pool(name="routep", bufs=2, space="PSUM") as routep:
                for i in range(NCH):
                    for h in range(2):
                        rt = routep.tile([128, M // 2], F32, tag="rt")
                        for mt in range(2):
                            o = h * 2 + mt
                            nc.tensor.matmul(
                                rt[:, mt * 512:(mt + 1) * 512],
                                wnormt_r[:, i * 128:(i + 1) * 128],
                                kbt_r[:, o * 512:(o + 1) * 512],
                                start=True, stop=True)
                        nc.scalar.activation(
                            out=er_t[:, i, h * (M // 2):(h + 1) * (M // 2)],
                            in_=rt, func=AF.Exp, bias=0.0, scale=inv_tau_w,
                            accum_out=rsum2[:, i, h:h + 1])
            rsum = work.tile([128, NCH], F32, tag="rsum")
            nc.vector.tensor_add(rsum, rsum2[:, :, 0], rsum2[:, :, 1])
            rphi = work.tile([128, NCH], F32, tag="rphi")
            nc.vector.reciprocal(rphi, rsum)
            phi = work.tile([128, NCH], F32, tag="phi")
            nc.vector.tensor_mul(phi, rphi, nov_t)
            for i in range(NCH):
                nc.vector.tensor_scalar(out=wt_t[:, i, 0:128], in0=wnorm[:, i],
                                        scalar1=phi[:, i:i + 1], scalar2=None,
                                        op0=ALU.mult)
                nc.vector.tensor_scalar(out=wt_t[:, i, 128:256], in0=wcand[:, i],
                                        scalar1=phi[:, i:i + 1], scalar2=None,
                                        op0=ALU.mult)
                nc.vector.tensor_copy(wt_t[:, i, 256:257], phi[:, i:i + 1])

            # ---------------- update matmuls: stage uK/uV/den/ssk ----------------
            uall = consts.tile([128, MCH, 257], F32)
            ssks = work.tile([128, MCH], F32, tag="ssks")
            al = work.tile([128, MCH], F32, tag="al")
            apv = work.tile([128, MCH], F32, tag="apv")
            oma = work.tile([128, MCH], F32, tag="oma")
            dng = work.tile([128, MCH], F32, tag="dng")
            rdn = work.tile([128, MCH], F32, tag="rdn")
            rdna = work.tile([128, MCH], F32, tag="rdna")
            nrmk = work.tile([128, MCH], F32, tag="nrmk")
            nrmkg = work.tile([128, MCH], F32, tag="nrmkg")
            rnk = work.tile([128, MCH], F32, tag="rnk")
            rna = work.tile([128, MCH], F32, tag="rna")
            spv = work.tile([128, MCH], F32, tag="spv")
            pres = work.tile([128, MCH], F32, tag="pres")
            nage = work.tile([128, MCH], F32, tag="nage")
            nks = consts.tile([128, MCH, D], F32)
            nvs = consts.tile([128, MCH, D], F32)
            uks = uall[:, :, 0:128]
            uvs = uall[:, :, 128:256]
            dens = uall[:, :, 256]
            GW = 4  # update group width (PSUM accumulators alive at once)
            with tc.tile_pool(name="updp", bufs=GW, space="PSUM") as updp:
                for grp in range(MCH // GW):
                    js = list(range(grp * GW, (grp + 1) * GW))
                    ups = {j: updp.tile([128, 257], F32, tag="up", name=f"up{j}") for j in js}
                    for i in range(NCH):
                        for j in js:
                            nc.tensor.matmul(ups[j],
                                             er_t[:, i, j * 128:(j + 1) * 128],
                                             wt_t[:, i], start=(i == 0),
                                             stop=(i == NCH - 1))
                    for j in js:
                        nc.vector.tensor_copy(uall[:, j], ups[j])
                        scrapu = blends.tile([128, D], F32, tag="scrapu")
                        nc.scalar.activation(out=scrapu, in_=ups[j][:, 0:128],
                                             func=AF.Square, bias=0.0, scale=1.0,
                                             accum_out=ssks[:, j:j + 1])
                    sl = slice(js[0], js[-1] + 1)
                    nc.vector.tensor_scalar(out=al[:, sl], in0=dens[:, sl],
                                            scalar1=gn_t, scalar2=1.0,
                                            op0=ALU.mult, op1=ALU.min)
                    nc.vector.tensor_mul(apv[:, sl], al[:, sl], act_t[:, sl])
                    nc.vector.tensor_scalar(out=oma[:, sl], in0=apv[:, sl],
                                            scalar1=-1.0, scalar2=1.0,
                                            op0=ALU.mult, op1=ALU.add)
                    nc.vector.tensor_scalar(out=dng[:, sl], in0=dens[:, sl],
                                            scalar1=1e-8, scalar2=None, op0=ALU.max)
                    nc.vector.reciprocal(rdn[:, sl], dng[:, sl])
                    nc.vector.tensor_mul(rdna[:, sl], rdn[:, sl], apv[:, sl])
                    nc.scalar.activation(out=nrmk[:, sl], in_=ssks[:, sl],
                                         func=AF.Sqrt, bias=0.0, scale=1.0)
                    nc.vector.tensor_scalar(out=nrmkg[:, sl], in0=nrmk[:, sl],
                                            scalar1=1e-12, scalar2=None, op0=ALU.max)
                    nc.vector.reciprocal(rnk[:, sl], nrmkg[:, sl])
                    nc.vector.tensor_mul(rna[:, sl], rnk[:, sl], apv[:, sl])
                    nc.vector.tensor_add(spv[:, sl], sb_t[:, sl], apv[:, sl])
                    nc.vector.tensor_scalar(out=pres[:, sl], in0=spv[:, sl],
                                            scalar1=0.0, scalar2=S_MAX,
                                            op0=ALU.max, op1=ALU.min)
                    nc.vector.tensor_mul(nage[:, sl], age_t[:, sl], oma[:, sl])
                    for j in js:
                        ek = blends.tile([128, D], F32, tag="ek")
                        nc.vector.tensor_scalar(out=ek, in0=uks[:, j],
                                                scalar1=rna[:, j:j + 1],
                                                scalar2=None, op0=ALU.mult)
                        fk = blends.tile([128, D], F32, tag="fk")
                        nc.gpsimd.tensor_scalar(out=fk, in0=kb_f[:, j],
                                                scalar1=oma[:, j:j + 1],
                                                scalar2=None, op0=ALU.mult)
                        nc.vector.tensor_add(nks[:, j], ek, fk)
                        ev = blends.tile([128, D], F32, tag="ev")
                        nc.vector.tensor_scalar(out=ev, in0=uvs[:, j],
                                                scalar1=rdna[:, j:j + 1],
                                                scalar2=None, op0=ALU.mult)
                        fv = blends.tile([128, D], F32, tag="fv")
                        nc.gpsimd.tensor_scalar(out=fv, in0=vb_f[:, j],
                                                scalar1=oma[:, j:j + 1],
                                                scalar2=None, op0=ALU.mult)
                        nc.gpsimd.tensor_add(nvs[:, j], ev, fv)
                nc.sync.dma_start(out=na_o[:, :], in_=nage)
                rows = work.tile([128, 1], F32, tag="rows")
                nc.vector.reduce_sum(rows, pres, axis=mybir.AxisListType.X)
                tot_ps = updp.tile([128, 1], F32, tag="tot")
                nc.tensor.matmul(tot_ps, ones_f, rows, start=True, stop=True)
                totg = work.tile([128, 1], F32, tag="totg")
                nc.vector.tensor_scalar(out=totg, in0=tot_ps, scalar1=1e-8,
                                        scalar2=None, op0=ALU.max)
            rtot = work.tile([128, 1], F32, tag="rtot")
            nc.vector.reciprocal(rtot, totg)
            sc32 = work.tile([128, 1], F32, tag="sc32")
            nc.vector.tensor_scalar(out=sc32, in0=rtot, scalar1=BUDGET,
                                    scalar2=1.0, op0=ALU.mult, op1=ALU.min)
            nss = work.tile([128, MCH], F32, tag="nss")
            nc.vector.tensor_scalar(out=nss, in0=pres, scalar1=sc32,
                                    scalar2=None, op0=ALU.mult)
            nc.sync.dma_start(out=ns_o[:, :], in_=nss)
            nc.sync.dma_start(out=nk_o[:, :, :], in_=nks)
            nc.sync.dma_start(out=nv_o[:, :, :], in_=nvs)

    nc.compile()
    return nc


def _softplus(x):
    return float(np.log1p(np.exp(-abs(x))) + max(x, 0.0))


def kernel(seed, w_cand, novelty, g_em, em_K, em_V, em_S, em_age,
           w1, w2, gate_bias, raw_tau, raw_tau_w, b):
    bi = int(b)
    seed = np.asarray(seed, np.float32)
    w_cand = np.asarray(w_cand, np.float32)
    novelty = np.asarray(novelty, np.float32)
    g_em = np.asarray(g_em, np.float32)
    Kb = np.asarray(em_K, np.float32)[:, bi]    # [BS, M, D]
    Vb = np.asarray(em_V, np.float32)[:, bi]
    Sb = np.asarray(em_S, np.float32)[:, bi]    # [BS, M]
    ageb = np.asarray(em_age, np.float32)[:, bi]
    w1b = np.asarray(w1, np.float32)[bi]        # [D]
    w2b = np.asarray(w2, np.float32)[bi]
    gbb = np.asarray(gate_bias, np.float32)[bi]
    tau = _softplus(float(np.asarray(raw_tau)[bi])) + 0.1
    tau_w = _softplus(float(np.asarray(raw_tau_w)[bi])) + 0.1

    key = (round(1.0 / tau, 9), round(1.0 / tau_w, 9))
    if key not in _CACHE:
        _CACHE[key] = _build(1.0 / tau, 1.0 / tau_w)
    nc = _CACHE[key]

    in_maps = []
    for s in range(BS):
        mb = np.where(Sb[s] > 0, 0.0, -1e30).astype(np.float32)
        act = (Sb[s] > 0).astype(np.float32)
        big_f = np.empty((128, PK_F_LEN), np.float32)
        big_f[:, PK_KB:PK_KB + MCH * D] = \
            Kb[s].reshape(MCH, 128, D).transpose(1, 0, 2).reshape(128, MCH * D)
        big_f[:, PK_VB:PK_VB + MCH * D] = \
            Vb[s].reshape(MCH, 128, D).transpose(1, 0, 2).reshape(128, MCH * D)
        big_f[:, PK_SEED:PK_SEED + N] = seed[s].T
        big_f[:, PK_WC:PK_WC + NCH * D] = \
            w_cand[s].reshape(NCH, 128, D).transpose(1, 0, 2).reshape(128, NCH * D)
        big_f[:, PK_NOV:PK_NOV + NCH] = novelty[s].reshape(NCH, 128).T
        big_f[:, PK_SB:PK_SB + MCH] = Sb[s].reshape(MCH, 128).T
        big_f[:, PK_AGE:PK_AGE + MCH] = ageb[s].reshape(MCH, 128).T
        big_f[:, PK_MB:PK_MB + MCH] = mb.reshape(MCH, 128).T
        big_f[:, PK_ACT:PK_ACT + MCH] = act.reshape(MCH, 128).T
        big_f[:, PK_GN] = float(g_em[s]) / N
        big_f[:, PK_W1] = w1b
        big_f[:, PK_W2] = w2b
        big_f[:, PK_GBH] = 0.5 * gbb
        big_r = np.empty((128, PKR_LEN), np.float32)
        big_r[:, PKR_KBT:PKR_KBT + M] = Kb[s].T
        big_r[:, PKR_VB:PKR_VB + MCH * D] = big_f[:, PK_VB:PK_VB + MCH * D]
        big_r[:, PKR_SEED:PKR_SEED + N] = seed[s].T
        in_maps.append({"big_f": big_f, "big_r": big_r})

    _trace = os.environ.get("KERNEL_TRACE", "0") == "1"
    res = run_bass_kernel_spmd(nc, in_maps, list(range(BS)), trace=_trace)
    if _trace and getattr(res, "exec_time_ns", None) is not None:
        print(f"HW exec time: {res.exec_time_ns} ns")

    y_em = np.empty((BS, N, D), np.float32)
    new_K = np.empty((BS, M, D), np.float32)
    new_V = np.empty((BS, M, D), np.float32)
    new_S = np.empty((BS, M), np.float32)
    new_age = np.empty((BS, M), np.float32)
    for s in range(BS):
        r = res.results[s]
        y_em[s] = r["yemt"].T
        new_K[s] = r["nk"].transpose(1, 0, 2).reshape(M, D)
        new_V[s] = r["nv"].transpose(1, 0, 2).reshape(M, D)
        new_S[s] = r["ns"].T.reshape(M)
        new_age[s] = r["na"].T.reshape(M)
    return (y_em, new_K, new_V, new_S, new_age)
